# revision 74
# baseline (speedup 1.0000x reference)
"""Trainium2 Bass kernel for nn_Decoder (mask-multiply + Linear(512->16) + overlap-add).

Full-input contract: kernel(mixture_w, est_mask, W) -> [4, 128008] float32.

Sharding: 8 cores = 4 batches x 2 K-halves (8000 frames each).

Folded overlap-add: out[8m+r] = sum_n est[n,m] W[r,n] + sum_n est[n,m-1] W[r+8,n],
so both overlap-add terms accumulate into ONE psum bank per chunk: the W[0:8]
("A") matmuls run against the chunk's est window and the W[8:16] ("B") matmuls
run against the same window shifted one frame left (each input block carries a
1-frame halo; block 0's halo is pre-zeroed by the host). No DVE add, no
second psum evacuation, no cross-chunk compute dependency.

The host packs each input block [mw; em] per-partition-contiguously, so every
input DMA is 128 descriptors of up to 32KB instead of 1024 small strided
ones; output pairs share one 8-transpose group so output DMAs write 256B
contiguous runs (1 descriptor per 8 frames). Total ~2.9K DMA descriptors per
pass vs ~19K for the naive layouts.

Per-chunk pipeline (all cross-engine consumer stages run >=1 chunk behind
their producer so no semaphore round-trip sits inside one chunk period):
  SP  : wt/ident DMAs, then one input DMA per block (14 blocks/pass: six
        2-chunk blocks then eight 1-chunk blocks -- big early hides the ramp,
        small late shortens the post-stream drain)
  DVE : est = x0 * x1 (f32r out) -- the only per-chunk DVE op
  PE  : 8 accumulating matmuls -> psO[8,w] (complete output block, j-major),
        then per output pair q at chunk 2q+2: 8 transposes res -> pst[qw/8,64]
  ACT : evac psO(g)->res, ct-copy pair (g-3)/2, output-DMA issue pair (g-4)/2
Tail (8 samples = W_B^T est[:,last]) rides the last chunk's matmul group.
Host adds the 8-sample seam between the two K-halves of each batch.

Semaphores incremented by MULTIPLE DMAs are rings (dsb/dss/osem): the 16
SDMA engines inc independently, so counts from back-to-back DMAs on one
semaphore interleave and a waiter could fire before the older DMA fully
landed; ring slots + a transitive gate order same-semaphore DMAs.
Every instruction carries at most one semaphore wait (ISA limit)."""

import numpy as np

import concourse.bass as bass
import concourse.mybir as mybir
from concourse.bass_utils import run_bass_kernel_spmd

F32 = mybir.dt.float32
F32R = mybir.dt.float32r

B, N, K, L = 4, 512, 16000, 16
STEP = L // 2              # 8
KLOC = K // 2              # 8000 frames per core
TLOC = STEP * (KLOC - 1) + L   # 64008 local output samples
# frames per chunk (<=500: psum bank; %4==0; count divisible by the xb/ct
# ring depths so bench-loop semaphore sites stay stable; the shrinking tail
# shortens the post-stream pipeline drain)
WIDTHS = [500] * 14 + [400, 200, 160, 120, 80, 40]
assert sum(WIDTHS) == KLOC and all(w % 4 == 0 and w <= 500 for w in WIDTHS)
WMAX = max(WIDTHS)
# output pairs: chunks (2q, 2q+1) share one transpose group of 8 -> the
# output DMA writes 64-float (256B) contiguous runs, 1 descriptor per
# 8 frames instead of per 4
NQ = len(WIDTHS) // 2
PQW = [WIDTHS[2 * q] + WIDTHS[2 * q + 1] for q in range(NQ)]
assert all(p % 8 == 0 and p // 8 <= 125 for p in PQW)
PWMAX = max(PQW)
# input DMA blocks: chunks 0-11 as six 2-chunk blocks (2-buffer ring),
# chunks 12-19 individually (4-buffer ring; the fine tail keeps the
# post-stream drain short). The host packs each block's [mw; em] data (incl.
# 1-frame halo; block 0's halo pre-zeroed) per-partition-contiguously, so
# every input DMA is 128 descriptors of up to 32KB instead of 1024 small
# strided ones.
NPAIRIN = 6                # 2-chunk input blocks (chunks 0..11)
NSING = 8                  # single-chunk input blocks (chunks 12..19)
NS_IN = NPAIRIN + NSING    # input blocks per pass
IN_HW = [1001] * NPAIRIN + [WIDTHS[c] + 1 for c in range(12, 20)]
DOFF = np.cumsum([0] + [8 * h for h in IN_HW]).tolist()
TOTX = DOFF[-1]


class _Waiter:
    """Absolute-target waits (single pass) or register-tracked targets with
    constant per-site deltas (inside a bench Fori hardware loop)."""

    def __init__(self, eng):
        self.eng = eng
        self.last = {}
        self.regs = None

    def wait_monotone(self, sem, target):
        """Clamp target up to this engine's previous wait on the same sem
        (for sites whose analytic target regresses; the stronger wait is
        safe when the producer chain never runs through this engine)."""
        if sem.name in self.last:
            target = max(target, self.last[sem.name][1])
        self.wait(sem, target)

    def wait(self, sem, target):
        if self.regs is None:
            self.eng.wait_ge(sem, target)
            self.last[sem.name] = (sem, target)
        else:
            _, prev = self.last[sem.name]
            delta = target - prev
            assert delta >= 0, (sem.name, prev, target)
            self.last[sem.name] = (sem, target)
            reg = self.regs[sem.name]
            if delta:
                self.eng.reg_add(reg, reg, delta)
            self.eng.wait_ge(sem, reg)

    def enter_loop(self):
        self.regs = {}
        for name, (sem, target) in self.last.items():
            reg = self.eng.alloc_register(f"{name}_tgt")
            self.eng.reg_mov(reg, target)
            self.regs[name] = reg


def _build(loops: int | None) -> bass.Bass:
    """loops=None -> graded single-pass kernel (absolute waits only).
    loops>=3 -> bench variant with per-engine Fori steady-state loops."""
    bench = loops is not None
    G = len(WIDTHS)                    # chunks per pass
    starts = np.cumsum([0] + WIDTHS).tolist()   # frame offset per chunk

    nc = bass.Bass()
    x = nc.dram_tensor("x", [128, TOTX], F32, kind="ExternalInput")
    # f32r has f32 storage: DMA the repacked weight straight into the f32r
    # stationary tile (full-rate PE) with no cast copy.
    wt = nc.dram_tensor("wt", [128, 4 * L], F32R, kind="ExternalInput")
    ident = nc.dram_tensor("ident", [8, 8], F32, kind="ExternalInput")
    out = nc.dram_tensor("out", [TLOC], F32, kind="ExternalOutput")

    wt_r = wt.rearrange("p (ni l) -> p ni l", ni=4)

    from contextlib import ExitStack

    with ExitStack() as stk:
        e = stk.enter_context
        xp = [e(nc.sbuf_tensor(f"xp{i}", [128, 8 * 1001], F32)) for i in range(2)]
        xss = [e(nc.sbuf_tensor(f"xss{i}", [128, 8 * 501], F32)) for i in range(4)]
        eb = [e(nc.sbuf_tensor(f"eb{i}", [128, 4, WMAX + 1], F32R)) for i in range(2)]
        wt_sb = e(nc.sbuf_tensor("wt_sb", [128, 4, L], F32R))
        id_sb = e(nc.sbuf_tensor("id_sb", [8, 8], F32))
        res = [e(nc.sbuf_tensor(f"res{i}", [8, PWMAX], F32)) for i in range(2)]
        res_tail = e(nc.sbuf_tensor("res_tail", [8, 1], F32))
        # ct ring: the out-DMA completion gate (osem) binds NCT pairs back,
        # so output DMAs queued behind multi-hundred-KB input-DMA packets
        # never stall the ACT pipeline (NQ % NCT == 0: bench site stability)
        NCT = 5
        ct = [e(nc.sbuf_tensor(f"ct{i}", [PWMAX // 8, 64], F32)) for i in range(NCT)]
        psO = [e(nc.psum_tensor(f"psO{i}", [8, WMAX], F32)) for i in range(2)]
        pst = [e(nc.psum_tensor(f"pst{i}", [PWMAX // 8, 64], F32)) for i in range(2)]
        psT = e(nc.psum_tensor("psT", [8, 2], F32))
        # Semaphores incremented by MULTIPLE DMAs must be rings: the 16 SDMA
        # engines inc independently, so counts from back-to-back DMAs on one
        # sem interleave and a waiter can fire before the older DMA fully
        # landed. Ring slot g%NX / g%NCT + a transitive gate (msem / the
        # ct-slot wait) orders same-sem DMAs.
        wsem = e(nc.semaphore("wsem"))
        dsb = [e(nc.semaphore(f"dsb{i}")) for i in range(2)]
        dss = [e(nc.semaphore(f"dss{i}")) for i in range(4)]
        msem = e(nc.semaphore("msem"))
        psem = e(nc.semaphore("psem"))
        tsem = e(nc.semaphore("tsem"))
        esem = e(nc.semaphore("esem"))
        ctsem = e(nc.semaphore("ctsem"))
        osem = [e(nc.semaphore(f"osem{i}")) for i in range(NCT)]
        osem_t = e(nc.semaphore("osem_t"))
        block = e(nc.Block())

        ET = mybir.EngineType

        # Semaphore ledger (g = global chunk index, c = g % G):
        #   mult(g) done  <=> msem = g + 1
        #   MMs(g) done   <=> psem = g + 1 + (tails of completed passes)
        #   evac(g) done  <=> esem = g + 1 + (tail evacs of completed passes)
        #   T(g) done     <=> tsem = g + 1
        #   ct(g) done    <=> ctsem = g + 1
        def psem_after_mm(g):
            return g + 1 + g // G

        def psem_after_tail(g):
            return g + 2 + g // G

        def esem_after_evac(g):
            return g + 1 + g // G

        def esem_after_tail(g):
            return g + 2 + g // G

        def loop_or_unroll(W, engine_type, chunk_fn, lo=0, hi=None):
            """Emit chunk_fn(lo..hi-1) unrolled (single pass), or peel two
            passes then Fori over the rest (bench)."""
            if not bench:
                for g in range(lo, hi if hi is not None else G):
                    chunk_fn(g)
                return
            for g in range(2 * G):
                chunk_fn(g)
            W.enter_loop()
            with nc.Fori(2, loops, engines=[engine_type]):
                for cc in range(G):
                    chunk_fn(2 * G + cc)

        @block.sync
        def _(sync):
            W = _Waiter(sync)

            def dma_block(it):
                p, j = it // NS_IN, it % NS_IN
                if j < NPAIRIN:    # 2-chunk block j (chunks 2j, 2j+1)
                    bi = NPAIRIN * p + j
                    if bi >= 2:
                        # xp[bi%2] free: mult of last chunk of block bi-2 done
                        gprev = G * ((bi - 2) // NPAIRIN) + 2 * ((bi - 2) % NPAIRIN) + 1
                        W.wait_monotone(msem, gprev + 1)
                    sync.dma_start(
                        xp[bi % 2][:], x[:, DOFF[j] : DOFF[j + 1]]
                    ).then_inc(dsb[bi % 2], 16)
                else:              # single-chunk block (chunk 12 + k)
                    k = j - NPAIRIN
                    si = NSING * p + k
                    if si >= 4:
                        # xss[si%4] free: mult of the chunk of block si-4 done
                        gprev = G * ((si - 4) // NSING) + 12 + (si - 4) % NSING
                        W.wait_monotone(msem, gprev + 1)
                    sync.dma_start(
                        xss[si % 4][:, 0 : 8 * IN_HW[j]], x[:, DOFF[j] : DOFF[j + 1]]
                    ).then_inc(dss[si % 4], 16)

            if not bench:
                for it in range(NS_IN):
                    dma_block(it)
            else:
                for it in range(2 * NS_IN):
                    dma_block(it)
                W.enter_loop()
                with nc.Fori(2, loops, engines=[ET.SP]):
                    for cc in range(NS_IN):
                        dma_block(2 * NS_IN + cc)
            if not bench:
                sync.wait_ge(esem, G + 1)   # tail evac done
                sync.dma_start(
                    out[STEP * KLOC : TLOC].rearrange("(p x) -> p x", x=1),
                    res_tail[:],
                ).then_inc(osem_t, 16)
                q_ep = NQ - 2
                qw = PQW[q_ep]
                pw = qw // 8
                f = starts[2 * q_ep]
                dst = out[8 * f : 8 * f + 8 * qw].rearrange(
                    "(p m j) -> p m j", p=pw, m=8
                )
                sync.wait_ge(ctsem, q_ep + 1)
                sync.dma_start(
                    dst,
                    ct[q_ep % NCT][0:pw, :].rearrange("p (m j) -> p m j", m=8),
                ).then_inc(osem[q_ep % NCT], 16)

        @block.vector
        def _(vector):
            W = _Waiter(vector)

            def chunk(g):
                c = g % G
                w = WIDTHS[c]
                b = g % 2
                p = g // G
                if c < 2 * NPAIRIN:
                    j = c // 2
                    bi = NPAIRIN * p + j
                    W.wait(dsb[bi % 2], 16 * (bi // 2 + 1))
                    buf, hw = xp[bi % 2], 1001
                    rel = 0 if c % 2 == 0 else WIDTHS[c - 1]
                else:
                    k = c - 2 * NPAIRIN
                    si = NSING * p + k
                    W.wait(dss[si % 4], 16 * (si // 4 + 1))
                    buf, hw = xss[si % 4], IN_HW[NPAIRIN + k]
                    rel = 0
                if g >= 2:
                    # eb[b] free: last read by MMs(g-2) (+ tail MMs if g-2
                    # ended a pass)
                    if (g - 2) % G == G - 1:
                        W.wait(psem, psem_after_tail(g - 2))
                    else:
                        W.wait(psem, psem_after_mm(g - 2))
                xv = buf[:, 0 : 8 * hw].rearrange(
                    "p (t ni k) -> p t ni k", t=2, ni=4
                )
                nc.vector.tensor_mul(
                    out=eb[b][:, :, 0 : w + 1],
                    in0=xv[:, 0, :, rel : rel + w + 1],
                    in1=xv[:, 1, :, rel : rel + w + 1],
                ).then_inc(msem, 1)

            loop_or_unroll(W, ET.DVE, chunk)

        @block.tensor
        def _(tensor):
            W = _Waiter(tensor)
            tensor.wait_ge(wsem, 32)   # wt_sb + id_sb loaded

            def transposes(q):
                # pair q = chunks (2q, 2q+1): 8 transposes res -> pst[PW, 64]
                b = q % 2
                qw = PQW[q % NQ]
                pw = qw // 8
                W.wait(esem, esem_after_evac(2 * q + 1))  # pair fully evac'd
                if q >= 2:
                    W.wait(ctsem, q - 1)           # pst[b] free (ct(q-2) done)
                for m in range(8):
                    tr = nc.tensor.transpose(
                        pst[b][0:pw, 8 * m : 8 * m + 8],
                        res[b][:, m:qw:8],
                        id_sb[:],
                    )
                    if m == 7:
                        tr.then_inc(tsem, 1)

            def chunk(g):
                c = g % G
                w = WIDTHS[c]
                b = g % 2
                W.wait(msem, g + 1)                    # mult(g) done
                if g >= 2:
                    W.wait(esem, esem_after_evac(g - 2))  # psO[b] free
                for ni in range(4):
                    nc.tensor.matmul(
                        psO[b][:, 0:w], wt_sb[:, ni, 0:STEP], eb[b][:, ni, 1 : w + 1],
                        start=(ni == 0), stop=False,
                    )
                for ni in range(4):
                    mm = nc.tensor.matmul(
                        psO[b][:, 0:w], wt_sb[:, ni, STEP:L], eb[b][:, ni, 0:w],
                        start=False, stop=(ni == 3),
                    )
                    if ni == 3:
                        mm.then_inc(psem, 1)
                if c == G - 1:
                    # tail: W_B^T est[:, last frame] -> psT[:, 1] (width 2:
                    # fp32r matmuls need an even moving width; col 0 is junk)
                    for ni in range(4):
                        mm = nc.tensor.matmul(
                            psT[:], wt_sb[:, ni, STEP:L], eb[b][:, ni, w - 1 : w + 1],
                            start=(ni == 0), stop=(ni == 3),
                        )
                        if ni == 3:
                            mm.then_inc(psem, 1)
                if g >= 2 and g % 2 == 0:
                    transposes((g - 2) // 2)

            loop_or_unroll(W, ET.PE, chunk)
            if not bench:
                transposes(NQ - 1)

        @block.scalar
        def _(scalar):
            W = _Waiter(scalar)
            scalar.dma_start(wt_sb[:], wt_r).then_inc(wsem, 16)
            scalar.dma_start(id_sb[:], ident[:]).then_inc(wsem, 16)

            def ct_copy(q):
                b = q % 2
                pw = PQW[q % NQ] // 8
                W.wait(tsem, q + 1)                # T(q) done
                if q >= NCT:
                    W.wait(osem[q % NCT], 16 * (q // NCT))  # ct slot free
                nc.scalar.copy(
                    out=ct[q % NCT][0:pw, :], in_=pst[b][0:pw, :]
                ).then_inc(ctsem, 1)

            def out_dma(q):
                qn = q % NQ
                qw = PQW[qn]
                pw = qw // 8
                f = starts[2 * qn]
                dst = out[8 * f : 8 * f + 8 * qw].rearrange(
                    "(p m j) -> p m j", p=pw, m=8
                )
                # the DMA trigger is async wrt the ACT pipe: gate on ctsem
                W.wait(ctsem, q + 1)
                scalar.dma_start(
                    dst, ct[q % NCT][0:pw, :].rearrange("p (m j) -> p m j", m=8)
                ).then_inc(osem[q % NCT], 16)

            def chunk(g):
                c = g % G
                w = WIDTHS[c]
                bq = (g // 2) % 2                  # res slot (per pair)
                off = 0 if g % 2 == 0 else WIDTHS[c - 1]
                W.wait(psem, psem_after_mm(g))
                if g >= 4:
                    W.wait(tsem, g // 2 - 1)       # res[bq] free (T(q-2) done)
                nc.scalar.copy(
                    out=res[bq][:, off : off + w], in_=psO[g % 2][:, 0:w]
                ).then_inc(esem, 1)
                if c == G - 1:   # tail evac (its completion wait overlaps below)
                    W.wait(psem, psem_after_tail(g))
                    W.wait(osem_t, 16 * (g // G))  # res_tail free (prev pass)
                    nc.scalar.copy(out=res_tail[:], in_=psT[:, 1:2]).then_inc(esem, 1)
                if g >= 3 and g % 2 == 1:
                    ct_copy((g - 3) // 2)
                if g >= 4 and g % 2 == 0:
                    out_dma((g - 4) // 2)
                if bench and c == G - 1:   # 32B tail output DMA (in-loop)
                    W.wait(esem, esem_after_tail(g))
                    scalar.dma_start(
                        out[STEP * KLOC : TLOC].rearrange("(p x) -> p x", x=1),
                        res_tail[:],
                    ).then_inc(osem_t, 16)

            loop_or_unroll(W, ET.Activation, chunk)
            if not bench:
                # final DMAs split across both rings: tail + pair NQ-2 issue
                # from the idle sync engine in parallel with ct/dma(NQ-1)
                ct_copy(NQ - 1)
                out_dma(NQ - 1)

    return nc


def build_nc():
    return _build(None)


def build_bench_nc(loops):
    return _build(loops)


def audit_waits(nc, max_show=12):
    """Count on_wait entries per instruction; the TPB ISA allows ONE."""
    import json

    d = json.loads(nc.to_json_bytes())
    bad = []

    def walk(blocks):
        for bb in blocks:
            for i in bb.get("instructions", []):
                si = i.get("sync_info") or {}
                w = si.get("on_wait") or []
                if len(w) > 1:
                    bad.append(
                        (
                            i["name"],
                            i.get("opcode"),
                            len(w),
                            [s_.get("ant_name") for s_ in w],
                        )
                    )
            walk(bb.get("blocks", []))

    walk(d["functions"][0]["blocks"])
    return bad[:max_show], len(bad)


_NC_CACHE = {}


def _get_nc():
    if "v3" not in _NC_CACHE:
        _NC_CACHE["v3"] = build_nc()
    return _NC_CACHE["v3"]


def _pack_x(xx):
    """xx [2, N, KLOC] -> flat [128, TOTX]: per input block, a
    per-partition-contiguous (t, ni, k) layout incl. the 1-frame halo
    (block 0's halo = zeros)."""
    flat = np.empty((128, TOTX), dtype=np.float32)
    starts_ = np.cumsum([0] + WIDTHS).tolist()
    bstart = [1000 * j for j in range(NPAIRIN)] + [starts_[c] for c in range(12, 20)]
    for i in range(NS_IN):
        f0, hw = bstart[i], IN_HW[i]
        blk = np.empty((2, N, hw), dtype=np.float32)
        if i == 0:
            blk[:, :, 0] = 0.0
            blk[:, :, 1:] = xx[:, :, 0 : hw - 1]
        else:
            blk[:] = xx[:, :, f0 - 1 : f0 + hw - 1]
        # [2, (ni p), hw] -> [p, (t ni hw)]
        flat[:, DOFF[i] : DOFF[i + 1]] = (
            blk.reshape(2, 4, 128, hw).transpose(2, 0, 1, 3).reshape(128, 8 * hw)
        )
    return flat


def make_in_maps(mixture_w, est_mask, W):
    mixture_w = np.asarray(mixture_w, dtype=np.float32)
    est_mask = np.asarray(est_mask, dtype=np.float32)
    W = np.asarray(W, dtype=np.float32)
    # wt[p, (ni l)] = W.T[ni*128+p, l]
    wtp = np.ascontiguousarray(
        W.T.reshape(4, 128, L).transpose(1, 0, 2).reshape(128, 4 * L)
    )
    ident = np.eye(8, dtype=np.float32)
    in_maps = []
    for c in range(8):
        b, h = c // 2, c % 2
        xx = np.stack(
            [
                mixture_w[b, :, h * KLOC : (h + 1) * KLOC],
                est_mask[b, :, h * KLOC : (h + 1) * KLOC],
            ]
        )
        in_maps.append({"x": _pack_x(xx), "wt": wtp, "ident": ident})
    return in_maps


def assemble(results):
    T = STEP * (K - 1) + L
    out = np.zeros((B, T), dtype=np.float32)
    for c in range(8):
        b, h = c // 2, c % 2
        out[b, h * STEP * KLOC : h * STEP * KLOC + TLOC] += results[c]["out"]
    return out


def run(mixture_w, est_mask, W, trace=False, **spmd_kwargs):
    """Shard, run on 8 cores, gather. Returns (out, BassKernelResults)."""
    in_maps = make_in_maps(mixture_w, est_mask, W)
    nc = _get_nc()
    kr = run_bass_kernel_spmd(
        nc, in_maps, core_ids=list(range(8)), trace=trace, **spmd_kwargs
    )
    return assemble(kr.results), kr


def kernel(mixture_w, est_mask, W):
    out, _ = run(mixture_w, est_mask, W)
    return out


# revision 76
# speedup vs baseline: 1.0706x; 1.0706x over previous
"""Trainium2 Bass kernel for nn_Decoder (mask-multiply + Linear(512->16) + overlap-add).

Full-input contract: kernel(mixture_w, est_mask, W) -> [4, 128008] float32.

Sharding: 8 cores = 4 batches x 2 K-halves (8000 frames each).

Folded overlap-add: out[8m+r] = sum_n est[n,m] W[r,n] + sum_n est[n,m-1] W[r+8,n],
so both overlap-add terms accumulate into ONE psum bank per chunk: the W[0:8]
("A") matmuls run against the chunk's est window and the W[8:16] ("B") matmuls
run against the same window shifted one frame left (each input block carries a
1-frame halo; block 0's halo is pre-zeroed by the host). No DVE add, no
second psum evacuation, no cross-chunk compute dependency.

The host packs each input block [mw; em] per-partition-contiguously, so every
input DMA is 128 descriptors of up to 32KB instead of 1024 small strided
ones; output pairs share one 8-transpose group so output DMAs write 256B
contiguous runs (1 descriptor per 8 frames). Total ~2.9K DMA descriptors per
pass vs ~19K for the naive layouts.

Per-chunk pipeline (all cross-engine consumer stages run >=1 chunk behind
their producer so no semaphore round-trip sits inside one chunk period):
  SP  : wt/ident DMAs, then one input DMA per block (14 blocks/pass: six
        2-chunk blocks then eight 1-chunk blocks -- big early hides the ramp,
        small late shortens the post-stream drain)
  DVE : est = x0 * x1 (f32r out) -- the only per-chunk DVE op
  PE  : 8 accumulating matmuls -> psO[8,w] (complete output block, j-major),
        then per output pair q at chunk 2q+2: 8 transposes res -> pst[qw/8,64]
  ACT : evac psO(g)->res, ct-copy pair (g-3)/2, output-DMA issue pair (g-4)/2
Tail (8 samples = W_B^T est[:,last]) rides the last chunk's matmul group.
Host adds the 8-sample seam between the two K-halves of each batch.

Semaphores incremented by MULTIPLE DMAs are rings (dsb/dss/osem): the 16
SDMA engines inc independently, so counts from back-to-back DMAs on one
semaphore interleave and a waiter could fire before the older DMA fully
landed; ring slots + a transitive gate order same-semaphore DMAs.
Every instruction carries at most one semaphore wait (ISA limit)."""

import numpy as np

import concourse.bass as bass
import concourse.mybir as mybir
from concourse.bass_utils import run_bass_kernel_spmd

F32 = mybir.dt.float32
F32R = mybir.dt.float32r

B, N, K, L = 4, 512, 16000, 16
STEP = L // 2              # 8
KLOC = K // 2              # 8000 frames per core
TLOC = STEP * (KLOC - 1) + L   # 64008 local output samples
# frames per chunk (<=512: psum bank; %16==0 pair sums for NT=16 transpose
# groups; count divisible by the ring depths so bench-loop semaphore sites
# stay stable; the shrinking tail shortens the post-stream pipeline drain)
WIDTHS = [512] * 14 + [320, 192, 128, 96, 64, 32]
assert sum(WIDTHS) == KLOC and all(w % 4 == 0 and w <= 512 for w in WIDTHS)
WMAX = max(WIDTHS)
# output pairs: chunks (2q, 2q+1) share one transpose group of NT=16 -> the
# output DMA writes 128-float (512B, line-rate) contiguous runs, 1
# descriptor per 16 frames
NT = 16
NQ = len(WIDTHS) // 2
PQW = [WIDTHS[2 * q] + WIDTHS[2 * q + 1] for q in range(NQ)]
assert all(p % NT == 0 and p // NT <= 128 for p in PQW)
PWMAX = max(PQW)
# input DMA blocks: chunks 0-11 as six 2-chunk blocks (2-buffer ring),
# chunks 12-19 individually (4-buffer ring; the fine tail keeps the
# post-stream drain short). The host packs each block's [mw; em] data (incl.
# 1-frame halo; block 0's halo pre-zeroed) per-partition-contiguously, so
# every input DMA is 128 descriptors of up to 33KB instead of 1024 small
# strided ones.
NPAIRIN = 6                # 2-chunk input blocks (chunks 0..11)
NSING = 8                  # single-chunk input blocks (chunks 12..19)
NS_IN = NPAIRIN + NSING    # input blocks per pass
IN_HW = [2 * 512 + 1] * NPAIRIN + [WIDTHS[c] + 1 for c in range(12, 20)]
DOFF = np.cumsum([0] + [8 * h for h in IN_HW]).tolist()
TOTX = DOFF[-1]


class _Waiter:
    """Absolute-target waits (single pass) or register-tracked targets with
    constant per-site deltas (inside a bench Fori hardware loop)."""

    def __init__(self, eng):
        self.eng = eng
        self.last = {}
        self.regs = None

    def wait_monotone(self, sem, target):
        """Clamp target up to this engine's previous wait on the same sem
        (for sites whose analytic target regresses; the stronger wait is
        safe when the producer chain never runs through this engine)."""
        if sem.name in self.last:
            target = max(target, self.last[sem.name][1])
        self.wait(sem, target)

    def wait(self, sem, target):
        if self.regs is None:
            self.eng.wait_ge(sem, target)
            self.last[sem.name] = (sem, target)
        else:
            _, prev = self.last[sem.name]
            delta = target - prev
            assert delta >= 0, (sem.name, prev, target)
            self.last[sem.name] = (sem, target)
            reg = self.regs[sem.name]
            if delta:
                self.eng.reg_add(reg, reg, delta)
            self.eng.wait_ge(sem, reg)

    def enter_loop(self):
        self.regs = {}
        for name, (sem, target) in self.last.items():
            reg = self.eng.alloc_register(f"{name}_tgt")
            self.eng.reg_mov(reg, target)
            self.regs[name] = reg


def _build(loops: int | None) -> bass.Bass:
    """loops=None -> graded single-pass kernel (absolute waits only).
    loops>=3 -> bench variant with per-engine Fori steady-state loops."""
    bench = loops is not None
    G = len(WIDTHS)                    # chunks per pass
    starts = np.cumsum([0] + WIDTHS).tolist()   # frame offset per chunk

    nc = bass.Bass()
    x = nc.dram_tensor("x", [128, TOTX], F32, kind="ExternalInput")
    # f32r has f32 storage: DMA the repacked weight straight into the f32r
    # stationary tile (full-rate PE) with no cast copy.
    wt = nc.dram_tensor("wt", [128, 4 * L], F32R, kind="ExternalInput")
    ident = nc.dram_tensor("ident", [8, 8], F32, kind="ExternalInput")
    out = nc.dram_tensor("out", [TLOC], F32, kind="ExternalOutput")

    wt_r = wt.rearrange("p (ni l) -> p ni l", ni=4)

    from contextlib import ExitStack

    with ExitStack() as stk:
        e = stk.enter_context
        xp = [e(nc.sbuf_tensor(f"xp{i}", [128, 8 * IN_HW[0]], F32)) for i in range(2)]
        xss = [e(nc.sbuf_tensor(f"xss{i}", [128, 8 * (WMAX + 1)], F32)) for i in range(4)]
        eb = [e(nc.sbuf_tensor(f"eb{i}", [128, 4, WMAX + 1], F32R)) for i in range(2)]
        wt_sb = e(nc.sbuf_tensor("wt_sb", [128, 4, L], F32R))
        id_sb = e(nc.sbuf_tensor("id_sb", [8, 8], F32))
        res = [e(nc.sbuf_tensor(f"res{i}", [8, PWMAX], F32)) for i in range(2)]
        res_tail = e(nc.sbuf_tensor("res_tail", [8, 1], F32))
        # ct ring: the out-DMA completion gate (osem) binds NCT pairs back,
        # so output DMAs queued behind multi-hundred-KB input-DMA packets
        # never stall the ACT pipeline (NQ % NCT == 0: bench site stability)
        NCT = 5
        ct = [e(nc.sbuf_tensor(f"ct{i}", [PWMAX // NT, 8 * NT], F32)) for i in range(NCT)]
        psO = [e(nc.psum_tensor(f"psO{i}", [8, WMAX], F32)) for i in range(2)]
        pst = [e(nc.psum_tensor(f"pst{i}", [PWMAX // NT, 8 * NT], F32)) for i in range(2)]
        psT = e(nc.psum_tensor("psT", [8, 2], F32))
        # Semaphores incremented by MULTIPLE DMAs must be rings: the 16 SDMA
        # engines inc independently, so counts from back-to-back DMAs on one
        # sem interleave and a waiter can fire before the older DMA fully
        # landed. Ring slot g%NX / g%NCT + a transitive gate (msem / the
        # ct-slot wait) orders same-sem DMAs.
        wsem = e(nc.semaphore("wsem"))
        dsb = [e(nc.semaphore(f"dsb{i}")) for i in range(2)]
        dss = [e(nc.semaphore(f"dss{i}")) for i in range(4)]
        msem = e(nc.semaphore("msem"))
        psem = e(nc.semaphore("psem"))
        tsem = e(nc.semaphore("tsem"))
        esem = e(nc.semaphore("esem"))
        ctsem = e(nc.semaphore("ctsem"))
        osem = [e(nc.semaphore(f"osem{i}")) for i in range(NCT)]
        osem_t = e(nc.semaphore("osem_t"))
        block = e(nc.Block())

        ET = mybir.EngineType

        # Semaphore ledger (g = global chunk index, c = g % G):
        #   mult(g) done  <=> msem = g + 1
        #   MMs(g) done   <=> psem = g + 1 + (tails of completed passes)
        #   evac(g) done  <=> esem = g + 1 + (tail evacs of completed passes)
        #   T(g) done     <=> tsem = g + 1
        #   ct(g) done    <=> ctsem = g + 1
        def psem_after_mm(g):
            return g + 1 + g // G

        def psem_after_tail(g):
            return g + 2 + g // G

        def esem_after_evac(g):
            return g + 1 + g // G

        def esem_after_tail(g):
            return g + 2 + g // G

        def loop_or_unroll(W, engine_type, chunk_fn, lo=0, hi=None):
            """Emit chunk_fn(lo..hi-1) unrolled (single pass), or peel two
            passes then Fori over the rest (bench)."""
            if not bench:
                for g in range(lo, hi if hi is not None else G):
                    chunk_fn(g)
                return
            for g in range(2 * G):
                chunk_fn(g)
            W.enter_loop()
            with nc.Fori(2, loops, engines=[engine_type]):
                for cc in range(G):
                    chunk_fn(2 * G + cc)

        @block.sync
        def _(sync):
            W = _Waiter(sync)

            def dma_block(it):
                p, j = it // NS_IN, it % NS_IN
                if j < NPAIRIN:    # 2-chunk block j (chunks 2j, 2j+1)
                    bi = NPAIRIN * p + j
                    if bi >= 2:
                        # xp[bi%2] free: mult of last chunk of block bi-2 done
                        gprev = G * ((bi - 2) // NPAIRIN) + 2 * ((bi - 2) % NPAIRIN) + 1
                        W.wait_monotone(msem, gprev + 1)
                    sync.dma_start(
                        xp[bi % 2][:], x[:, DOFF[j] : DOFF[j + 1]]
                    ).then_inc(dsb[bi % 2], 16)
                else:              # single-chunk block (chunk 12 + k)
                    k = j - NPAIRIN
                    si = NSING * p + k
                    if si >= 4:
                        # xss[si%4] free: mult of the chunk of block si-4 done
                        gprev = G * ((si - 4) // NSING) + 12 + (si - 4) % NSING
                        W.wait_monotone(msem, gprev + 1)
                    sync.dma_start(
                        xss[si % 4][:, 0 : 8 * IN_HW[j]], x[:, DOFF[j] : DOFF[j + 1]]
                    ).then_inc(dss[si % 4], 16)

            if not bench:
                for it in range(NS_IN):
                    dma_block(it)
            else:
                for it in range(2 * NS_IN):
                    dma_block(it)
                W.enter_loop()
                with nc.Fori(2, loops, engines=[ET.SP]):
                    for cc in range(NS_IN):
                        dma_block(2 * NS_IN + cc)
            if not bench:
                sync.wait_ge(esem, G + 1)   # tail evac done
                sync.dma_start(
                    out[STEP * KLOC : TLOC].rearrange("(p x) -> p x", x=1),
                    res_tail[:],
                ).then_inc(osem_t, 16)
                q_ep = NQ - 2
                qw = PQW[q_ep]
                pw = qw // NT
                f = starts[2 * q_ep]
                dst = out[8 * f : 8 * f + 8 * qw].rearrange(
                    "(p m j) -> p m j", p=pw, m=NT
                )
                sync.wait_ge(ctsem, q_ep + 1)
                sync.dma_start(
                    dst,
                    ct[q_ep % NCT][0:pw, :].rearrange("p (m j) -> p m j", m=NT),
                ).then_inc(osem[q_ep % NCT], 16)

        @block.vector
        def _(vector):
            W = _Waiter(vector)

            def chunk(g):
                c = g % G
                w = WIDTHS[c]
                b = g % 2
                p = g // G
                if c < 2 * NPAIRIN:
                    j = c // 2
                    bi = NPAIRIN * p + j
                    W.wait(dsb[bi % 2], 16 * (bi // 2 + 1))
                    buf, hw = xp[bi % 2], IN_HW[j]
                    rel = 0 if c % 2 == 0 else WIDTHS[c - 1]
                else:
                    k = c - 2 * NPAIRIN
                    si = NSING * p + k
                    W.wait(dss[si % 4], 16 * (si // 4 + 1))
                    buf, hw = xss[si % 4], IN_HW[NPAIRIN + k]
                    rel = 0
                if g >= 2:
                    # eb[b] free: last read by MMs(g-2) (+ tail MMs if g-2
                    # ended a pass)
                    if (g - 2) % G == G - 1:
                        W.wait(psem, psem_after_tail(g - 2))
                    else:
                        W.wait(psem, psem_after_mm(g - 2))
                xv = buf[:, 0 : 8 * hw].rearrange(
                    "p (t ni k) -> p t ni k", t=2, ni=4
                )
                nc.vector.tensor_mul(
                    out=eb[b][:, :, 0 : w + 1],
                    in0=xv[:, 0, :, rel : rel + w + 1],
                    in1=xv[:, 1, :, rel : rel + w + 1],
                ).then_inc(msem, 1)

            loop_or_unroll(W, ET.DVE, chunk)

        @block.tensor
        def _(tensor):
            W = _Waiter(tensor)
            tensor.wait_ge(wsem, 32)   # wt_sb + id_sb loaded

            def transposes(q):
                # pair q = chunks (2q, 2q+1): 8 transposes res -> pst[PW, 64]
                b = q % 2
                qw = PQW[q % NQ]
                pw = qw // NT
                W.wait(esem, esem_after_evac(2 * q + 1))  # pair fully evac'd
                if q >= 2:
                    W.wait(ctsem, q - 1)           # pst[b] free (ct(q-2) done)
                for m in range(NT):
                    tr = nc.tensor.transpose(
                        pst[b][0:pw, 8 * m : 8 * m + 8],
                        res[b][:, m:qw:NT],
                        id_sb[:],
                    )
                    if m == NT - 1:
                        tr.then_inc(tsem, 1)

            def chunk(g):
                c = g % G
                w = WIDTHS[c]
                b = g % 2
                W.wait(msem, g + 1)                    # mult(g) done
                if g >= 2:
                    W.wait(esem, esem_after_evac(g - 2))  # psO[b] free
                for ni in range(4):
                    nc.tensor.matmul(
                        psO[b][:, 0:w], wt_sb[:, ni, 0:STEP], eb[b][:, ni, 1 : w + 1],
                        start=(ni == 0), stop=False,
                    )
                for ni in range(4):
                    mm = nc.tensor.matmul(
                        psO[b][:, 0:w], wt_sb[:, ni, STEP:L], eb[b][:, ni, 0:w],
                        start=False, stop=(ni == 3),
                    )
                    if ni == 3:
                        mm.then_inc(psem, 1)
                if c == G - 1:
                    # tail: W_B^T est[:, last frame] -> psT[:, 1] (width 2:
                    # fp32r matmuls need an even moving width; col 0 is junk)
                    for ni in range(4):
                        mm = nc.tensor.matmul(
                            psT[:], wt_sb[:, ni, STEP:L], eb[b][:, ni, w - 1 : w + 1],
                            start=(ni == 0), stop=(ni == 3),
                        )
                        if ni == 3:
                            mm.then_inc(psem, 1)
                if g >= 2 and g % 2 == 0:
                    transposes((g - 2) // 2)

            loop_or_unroll(W, ET.PE, chunk)
            if not bench:
                transposes(NQ - 1)

        @block.scalar
        def _(scalar):
            W = _Waiter(scalar)
            scalar.dma_start(wt_sb[:], wt_r).then_inc(wsem, 16)
            scalar.dma_start(id_sb[:], ident[:]).then_inc(wsem, 16)

            def ct_copy(q):
                b = q % 2
                pw = PQW[q % NQ] // NT
                W.wait(tsem, q + 1)                # T(q) done
                if q >= NCT:
                    W.wait(osem[q % NCT], 16 * (q // NCT))  # ct slot free
                nc.scalar.copy(
                    out=ct[q % NCT][0:pw, :], in_=pst[b][0:pw, :]
                ).then_inc(ctsem, 1)

            def out_dma(q):
                qn = q % NQ
                qw = PQW[qn]
                pw = qw // NT
                f = starts[2 * qn]
                dst = out[8 * f : 8 * f + 8 * qw].rearrange(
                    "(p m j) -> p m j", p=pw, m=NT
                )
                # the DMA trigger is async wrt the ACT pipe: gate on ctsem
                W.wait(ctsem, q + 1)
                scalar.dma_start(
                    dst, ct[q % NCT][0:pw, :].rearrange("p (m j) -> p m j", m=NT)
                ).then_inc(osem[q % NCT], 16)

            def chunk(g):
                c = g % G
                w = WIDTHS[c]
                bq = (g // 2) % 2                  # res slot (per pair)
                off = 0 if g % 2 == 0 else WIDTHS[c - 1]
                W.wait(psem, psem_after_mm(g))
                if g >= 4:
                    W.wait(tsem, g // 2 - 1)       # res[bq] free (T(q-2) done)
                nc.scalar.copy(
                    out=res[bq][:, off : off + w], in_=psO[g % 2][:, 0:w]
                ).then_inc(esem, 1)
                if c == G - 1:   # tail evac (its completion wait overlaps below)
                    W.wait(psem, psem_after_tail(g))
                    W.wait(osem_t, 16 * (g // G))  # res_tail free (prev pass)
                    nc.scalar.copy(out=res_tail[:], in_=psT[:, 1:2]).then_inc(esem, 1)
                if g >= 3 and g % 2 == 1:
                    ct_copy((g - 3) // 2)
                if g >= 4 and g % 2 == 0:
                    out_dma((g - 4) // 2)
                if bench and c == G - 1:   # 32B tail output DMA (in-loop)
                    W.wait(esem, esem_after_tail(g))
                    scalar.dma_start(
                        out[STEP * KLOC : TLOC].rearrange("(p x) -> p x", x=1),
                        res_tail[:],
                    ).then_inc(osem_t, 16)

            loop_or_unroll(W, ET.Activation, chunk)
            if not bench:
                # final DMAs split across both rings: tail + pair NQ-2 issue
                # from the idle sync engine in parallel with ct/dma(NQ-1)
                ct_copy(NQ - 1)
                out_dma(NQ - 1)

    return nc


def build_nc():
    return _build(None)


def build_bench_nc(loops):
    return _build(loops)


def audit_waits(nc, max_show=12):
    """Count on_wait entries per instruction; the TPB ISA allows ONE."""
    import json

    d = json.loads(nc.to_json_bytes())
    bad = []

    def walk(blocks):
        for bb in blocks:
            for i in bb.get("instructions", []):
                si = i.get("sync_info") or {}
                w = si.get("on_wait") or []
                if len(w) > 1:
                    bad.append(
                        (
                            i["name"],
                            i.get("opcode"),
                            len(w),
                            [s_.get("ant_name") for s_ in w],
                        )
                    )
            walk(bb.get("blocks", []))

    walk(d["functions"][0]["blocks"])
    return bad[:max_show], len(bad)


_NC_CACHE = {}


def _get_nc():
    if "v3" not in _NC_CACHE:
        _NC_CACHE["v3"] = build_nc()
    return _NC_CACHE["v3"]


def _pack_x(xx):
    """xx [2, N, KLOC] -> flat [128, TOTX]: per input block, a
    per-partition-contiguous (t, ni, k) layout incl. the 1-frame halo
    (block 0's halo = zeros)."""
    flat = np.empty((128, TOTX), dtype=np.float32)
    starts_ = np.cumsum([0] + WIDTHS).tolist()
    bstart = [1024 * j for j in range(NPAIRIN)] + [starts_[c] for c in range(12, 20)]
    for i in range(NS_IN):
        f0, hw = bstart[i], IN_HW[i]
        blk = np.empty((2, N, hw), dtype=np.float32)
        if i == 0:
            blk[:, :, 0] = 0.0
            blk[:, :, 1:] = xx[:, :, 0 : hw - 1]
        else:
            blk[:] = xx[:, :, f0 - 1 : f0 + hw - 1]
        # [2, (ni p), hw] -> [p, (t ni hw)]
        flat[:, DOFF[i] : DOFF[i + 1]] = (
            blk.reshape(2, 4, 128, hw).transpose(2, 0, 1, 3).reshape(128, 8 * hw)
        )
    return flat


def make_in_maps(mixture_w, est_mask, W):
    mixture_w = np.asarray(mixture_w, dtype=np.float32)
    est_mask = np.asarray(est_mask, dtype=np.float32)
    W = np.asarray(W, dtype=np.float32)
    # wt[p, (ni l)] = W.T[ni*128+p, l]
    wtp = np.ascontiguousarray(
        W.T.reshape(4, 128, L).transpose(1, 0, 2).reshape(128, 4 * L)
    )
    ident = np.eye(8, dtype=np.float32)
    in_maps = []
    for c in range(8):
        b, h = c // 2, c % 2
        xx = np.stack(
            [
                mixture_w[b, :, h * KLOC : (h + 1) * KLOC],
                est_mask[b, :, h * KLOC : (h + 1) * KLOC],
            ]
        )
        in_maps.append({"x": _pack_x(xx), "wt": wtp, "ident": ident})
    return in_maps


def assemble(results):
    T = STEP * (K - 1) + L
    out = np.zeros((B, T), dtype=np.float32)
    for c in range(8):
        b, h = c // 2, c % 2
        out[b, h * STEP * KLOC : h * STEP * KLOC + TLOC] += results[c]["out"]
    return out


def run(mixture_w, est_mask, W, trace=False, **spmd_kwargs):
    """Shard, run on 8 cores, gather. Returns (out, BassKernelResults)."""
    in_maps = make_in_maps(mixture_w, est_mask, W)
    nc = _get_nc()
    kr = run_bass_kernel_spmd(
        nc, in_maps, core_ids=list(range(8)), trace=trace, **spmd_kwargs
    )
    return assemble(kr.results), kr


def kernel(mixture_w, est_mask, W):
    out, _ = run(mixture_w, est_mask, W)
    return out


# revision 77
# speedup vs baseline: 1.0873x; 1.0156x over previous
"""Trainium2 Bass kernel for nn_Decoder (mask-multiply + Linear(512->16) + overlap-add).

Full-input contract: kernel(mixture_w, est_mask, W) -> [4, 128008] float32.

Sharding: 8 cores = 4 batches x 2 K-halves (8000 frames each).

Folded overlap-add: out[8m+r] = sum_n est[n,m] W[r,n] + sum_n est[n,m-1] W[r+8,n],
so both overlap-add terms accumulate into ONE psum bank per chunk: the W[0:8]
("A") matmuls run against the chunk's est window and the W[8:16] ("B") matmuls
run against the same window shifted one frame left (each input block carries a
1-frame halo; block 0's halo is pre-zeroed by the host). No DVE add, no
second psum evacuation, no cross-chunk compute dependency.

The host packs each input block [mw; em] per-partition-contiguously, so every
input DMA is 128 descriptors of up to 32KB instead of 1024 small strided
ones; output pairs share one 8-transpose group so output DMAs write 256B
contiguous runs (1 descriptor per 8 frames). Total ~2.9K DMA descriptors per
pass vs ~19K for the naive layouts.

Per-chunk pipeline (all cross-engine consumer stages run >=1 chunk behind
their producer so no semaphore round-trip sits inside one chunk period):
  SP  : wt/ident DMAs, then one input DMA per block (14 blocks/pass: six
        2-chunk blocks then eight 1-chunk blocks -- big early hides the ramp,
        small late shortens the post-stream drain)
  DVE : est = x0 * x1 (f32r out) -- the only per-chunk DVE op
  PE  : 8 accumulating matmuls -> psO[8,w] (complete output block, j-major),
        then per output pair q at chunk 2q+2: 8 transposes res -> pst[qw/8,64]
  ACT : evac psO(g)->res, ct-copy pair (g-3)/2, output-DMA issue pair (g-4)/2
Tail (8 samples = W_B^T est[:,last]) rides the last chunk's matmul group.
Host adds the 8-sample seam between the two K-halves of each batch.

Semaphores incremented by MULTIPLE DMAs are rings (dsb/dss/osem): the 16
SDMA engines inc independently, so counts from back-to-back DMAs on one
semaphore interleave and a waiter could fire before the older DMA fully
landed; ring slots + a transitive gate order same-semaphore DMAs.
Every instruction carries at most one semaphore wait (ISA limit)."""

import numpy as np

import concourse.bass as bass
import concourse.mybir as mybir
from concourse.bass_utils import run_bass_kernel_spmd

F32 = mybir.dt.float32
F32R = mybir.dt.float32r

B, N, K, L = 4, 512, 16000, 16
STEP = L // 2              # 8
KLOC = K // 2              # 8000 frames per core
TLOC = STEP * (KLOC - 1) + L   # 64008 local output samples
# frames per chunk (<=512: psum bank; %16==0 pair sums for NT=16 transpose
# groups; count divisible by the ring depths so bench-loop semaphore sites
# stay stable; the shrinking tail shortens the post-stream pipeline drain)
WIDTHS = [512] * 14 + [320, 192, 128, 96, 64, 32]
assert sum(WIDTHS) == KLOC and all(w % 4 == 0 and w <= 512 for w in WIDTHS)
WMAX = max(WIDTHS)
# output pairs: chunks (2q, 2q+1) share one transpose group of NT=32 -> the
# output DMA writes 256-float (1KB, line-rate) contiguous runs, 1
# descriptor per 32 frames
NT = 32
NQ = len(WIDTHS) // 2
PQW = [WIDTHS[2 * q] + WIDTHS[2 * q + 1] for q in range(NQ)]
assert all(p % NT == 0 and p // NT <= 128 for p in PQW)
PWMAX = max(PQW)
# input DMA blocks: chunks 0-11 as six 2-chunk blocks (2-buffer ring),
# chunks 12-19 individually (4-buffer ring; the fine tail keeps the
# post-stream drain short). The host packs each block's [mw; em] data (incl.
# 1-frame halo; block 0's halo pre-zeroed) per-partition-contiguously, so
# every input DMA is 128 descriptors of up to 33KB instead of 1024 small
# strided ones.
NPAIRIN = 6                # 2-chunk input blocks (chunks 0..11)
NSING = 8                  # single-chunk input blocks (chunks 12..19)
NS_IN = NPAIRIN + NSING    # input blocks per pass
IN_HW = [2 * 512 + 1] * NPAIRIN + [WIDTHS[c] + 1 for c in range(12, 20)]
DOFF = np.cumsum([0] + [8 * h for h in IN_HW]).tolist()
TOTX = DOFF[-1]


class _Waiter:
    """Absolute-target waits (single pass) or register-tracked targets with
    constant per-site deltas (inside a bench Fori hardware loop)."""

    def __init__(self, eng):
        self.eng = eng
        self.last = {}
        self.regs = None

    def wait_monotone(self, sem, target):
        """Clamp target up to this engine's previous wait on the same sem
        (for sites whose analytic target regresses; the stronger wait is
        safe when the producer chain never runs through this engine)."""
        if sem.name in self.last:
            target = max(target, self.last[sem.name][1])
        self.wait(sem, target)

    def wait(self, sem, target):
        if self.regs is None:
            self.eng.wait_ge(sem, target)
            self.last[sem.name] = (sem, target)
        else:
            _, prev = self.last[sem.name]
            delta = target - prev
            assert delta >= 0, (sem.name, prev, target)
            self.last[sem.name] = (sem, target)
            reg = self.regs[sem.name]
            if delta:
                self.eng.reg_add(reg, reg, delta)
            self.eng.wait_ge(sem, reg)

    def enter_loop(self):
        self.regs = {}
        for name, (sem, target) in self.last.items():
            reg = self.eng.alloc_register(f"{name}_tgt")
            self.eng.reg_mov(reg, target)
            self.regs[name] = reg


def _build(loops: int | None) -> bass.Bass:
    """loops=None -> graded single-pass kernel (absolute waits only).
    loops>=3 -> bench variant with per-engine Fori steady-state loops."""
    bench = loops is not None
    G = len(WIDTHS)                    # chunks per pass
    starts = np.cumsum([0] + WIDTHS).tolist()   # frame offset per chunk

    nc = bass.Bass()
    x = nc.dram_tensor("x", [128, TOTX], F32, kind="ExternalInput")
    # f32r has f32 storage: DMA the repacked weight straight into the f32r
    # stationary tile (full-rate PE) with no cast copy.
    wt = nc.dram_tensor("wt", [128, 4 * L], F32R, kind="ExternalInput")
    ident = nc.dram_tensor("ident", [8, 8], F32, kind="ExternalInput")
    out = nc.dram_tensor("out", [TLOC], F32, kind="ExternalOutput")

    wt_r = wt.rearrange("p (ni l) -> p ni l", ni=4)

    from contextlib import ExitStack

    with ExitStack() as stk:
        e = stk.enter_context
        xp = [e(nc.sbuf_tensor(f"xp{i}", [128, 8 * IN_HW[0]], F32)) for i in range(2)]
        xss = [e(nc.sbuf_tensor(f"xss{i}", [128, 8 * (WMAX + 1)], F32)) for i in range(4)]
        eb = [e(nc.sbuf_tensor(f"eb{i}", [128, 4, WMAX + 1], F32R)) for i in range(2)]
        wt_sb = e(nc.sbuf_tensor("wt_sb", [128, 4, L], F32R))
        id_sb = e(nc.sbuf_tensor("id_sb", [8, 8], F32))
        res = [e(nc.sbuf_tensor(f"res{i}", [8, PWMAX], F32)) for i in range(2)]
        res_tail = e(nc.sbuf_tensor("res_tail", [8, 1], F32))
        # ct ring: the out-DMA completion gate (osem) binds NCT pairs back,
        # so output DMAs queued behind multi-hundred-KB input-DMA packets
        # never stall the ACT pipeline (NQ % NCT == 0: bench site stability)
        NCT = 5
        ct = [e(nc.sbuf_tensor(f"ct{i}", [PWMAX // NT, 8 * NT], F32)) for i in range(NCT)]
        psO = [e(nc.psum_tensor(f"psO{i}", [8, WMAX], F32)) for i in range(2)]
        pst = [e(nc.psum_tensor(f"pst{i}", [PWMAX // NT, 8 * NT], F32)) for i in range(2)]
        psT = e(nc.psum_tensor("psT", [8, 2], F32))
        # Semaphores incremented by MULTIPLE DMAs must be rings: the 16 SDMA
        # engines inc independently, so counts from back-to-back DMAs on one
        # sem interleave and a waiter can fire before the older DMA fully
        # landed. Ring slot g%NX / g%NCT + a transitive gate (msem / the
        # ct-slot wait) orders same-sem DMAs.
        wsem = e(nc.semaphore("wsem"))
        dsb = [e(nc.semaphore(f"dsb{i}")) for i in range(2)]
        dss = [e(nc.semaphore(f"dss{i}")) for i in range(4)]
        msem = e(nc.semaphore("msem"))
        psem = e(nc.semaphore("psem"))
        tsem = e(nc.semaphore("tsem"))
        esem = e(nc.semaphore("esem"))
        ctsem = e(nc.semaphore("ctsem"))
        osem = [e(nc.semaphore(f"osem{i}")) for i in range(NCT)]
        osem_t = e(nc.semaphore("osem_t"))
        block = e(nc.Block())

        ET = mybir.EngineType

        # Semaphore ledger (g = global chunk index, c = g % G):
        #   mult(g) done  <=> msem = g + 1
        #   MMs(g) done   <=> psem = g + 1 + (tails of completed passes)
        #   evac(g) done  <=> esem = g + 1 + (tail evacs of completed passes)
        #   T(g) done     <=> tsem = g + 1
        #   ct(g) done    <=> ctsem = g + 1
        def psem_after_mm(g):
            return g + 1 + g // G

        def psem_after_tail(g):
            return g + 2 + g // G

        def esem_after_evac(g):
            return g + 1 + g // G

        def esem_after_tail(g):
            return g + 2 + g // G

        def loop_or_unroll(W, engine_type, chunk_fn, lo=0, hi=None):
            """Emit chunk_fn(lo..hi-1) unrolled (single pass), or peel two
            passes then Fori over the rest (bench)."""
            if not bench:
                for g in range(lo, hi if hi is not None else G):
                    chunk_fn(g)
                return
            for g in range(2 * G):
                chunk_fn(g)
            W.enter_loop()
            with nc.Fori(2, loops, engines=[engine_type]):
                for cc in range(G):
                    chunk_fn(2 * G + cc)

        @block.sync
        def _(sync):
            W = _Waiter(sync)

            def dma_block(it):
                p, j = it // NS_IN, it % NS_IN
                if j < NPAIRIN:    # 2-chunk block j (chunks 2j, 2j+1)
                    bi = NPAIRIN * p + j
                    if bi >= 2:
                        # xp[bi%2] free: mult of last chunk of block bi-2 done
                        gprev = G * ((bi - 2) // NPAIRIN) + 2 * ((bi - 2) % NPAIRIN) + 1
                        W.wait_monotone(msem, gprev + 1)
                    sync.dma_start(
                        xp[bi % 2][:], x[:, DOFF[j] : DOFF[j + 1]]
                    ).then_inc(dsb[bi % 2], 16)
                else:              # single-chunk block (chunk 12 + k)
                    k = j - NPAIRIN
                    si = NSING * p + k
                    if si >= 4:
                        # xss[si%4] free: mult of the chunk of block si-4 done
                        gprev = G * ((si - 4) // NSING) + 12 + (si - 4) % NSING
                        W.wait_monotone(msem, gprev + 1)
                    sync.dma_start(
                        xss[si % 4][:, 0 : 8 * IN_HW[j]], x[:, DOFF[j] : DOFF[j + 1]]
                    ).then_inc(dss[si % 4], 16)

            if not bench:
                for it in range(NS_IN):
                    dma_block(it)
            else:
                for it in range(2 * NS_IN):
                    dma_block(it)
                W.enter_loop()
                with nc.Fori(2, loops, engines=[ET.SP]):
                    for cc in range(NS_IN):
                        dma_block(2 * NS_IN + cc)
            if not bench:
                sync.wait_ge(esem, G + 1)   # tail evac done
                sync.dma_start(
                    out[STEP * KLOC : TLOC].rearrange("(p x) -> p x", x=1),
                    res_tail[:],
                ).then_inc(osem_t, 16)
                q_ep = NQ - 2
                qw = PQW[q_ep]
                pw = qw // NT
                f = starts[2 * q_ep]
                dst = out[8 * f : 8 * f + 8 * qw].rearrange(
                    "(p m j) -> p m j", p=pw, m=NT
                )
                sync.wait_ge(ctsem, q_ep + 1)
                sync.dma_start(
                    dst,
                    ct[q_ep % NCT][0:pw, :].rearrange("p (m j) -> p m j", m=NT),
                ).then_inc(osem[q_ep % NCT], 16)

        @block.vector
        def _(vector):
            W = _Waiter(vector)

            def chunk(g):
                c = g % G
                w = WIDTHS[c]
                b = g % 2
                p = g // G
                if c < 2 * NPAIRIN:
                    j = c // 2
                    bi = NPAIRIN * p + j
                    W.wait(dsb[bi % 2], 16 * (bi // 2 + 1))
                    buf, hw = xp[bi % 2], IN_HW[j]
                    rel = 0 if c % 2 == 0 else WIDTHS[c - 1]
                else:
                    k = c - 2 * NPAIRIN
                    si = NSING * p + k
                    W.wait(dss[si % 4], 16 * (si // 4 + 1))
                    buf, hw = xss[si % 4], IN_HW[NPAIRIN + k]
                    rel = 0
                if g >= 2:
                    # eb[b] free: last read by MMs(g-2) (+ tail MMs if g-2
                    # ended a pass)
                    if (g - 2) % G == G - 1:
                        W.wait(psem, psem_after_tail(g - 2))
                    else:
                        W.wait(psem, psem_after_mm(g - 2))
                xv = buf[:, 0 : 8 * hw].rearrange(
                    "p (t ni k) -> p t ni k", t=2, ni=4
                )
                nc.vector.tensor_mul(
                    out=eb[b][:, :, 0 : w + 1],
                    in0=xv[:, 0, :, rel : rel + w + 1],
                    in1=xv[:, 1, :, rel : rel + w + 1],
                ).then_inc(msem, 1)

            loop_or_unroll(W, ET.DVE, chunk)

        @block.tensor
        def _(tensor):
            W = _Waiter(tensor)
            tensor.wait_ge(wsem, 32)   # wt_sb + id_sb loaded

            def transposes(q):
                # pair q = chunks (2q, 2q+1): 8 transposes res -> pst[PW, 64]
                b = q % 2
                qw = PQW[q % NQ]
                pw = qw // NT
                W.wait(esem, esem_after_evac(2 * q + 1))  # pair fully evac'd
                if q >= 2:
                    W.wait(ctsem, q - 1)           # pst[b] free (ct(q-2) done)
                for m in range(NT):
                    tr = nc.tensor.transpose(
                        pst[b][0:pw, 8 * m : 8 * m + 8],
                        res[b][:, m:qw:NT],
                        id_sb[:],
                    )
                    if m == NT - 1:
                        tr.then_inc(tsem, 1)

            def chunk(g):
                c = g % G
                w = WIDTHS[c]
                b = g % 2
                W.wait(msem, g + 1)                    # mult(g) done
                if g >= 2:
                    W.wait(esem, esem_after_evac(g - 2))  # psO[b] free
                for ni in range(4):
                    nc.tensor.matmul(
                        psO[b][:, 0:w], wt_sb[:, ni, 0:STEP], eb[b][:, ni, 1 : w + 1],
                        start=(ni == 0), stop=False,
                    )
                for ni in range(4):
                    mm = nc.tensor.matmul(
                        psO[b][:, 0:w], wt_sb[:, ni, STEP:L], eb[b][:, ni, 0:w],
                        start=False, stop=(ni == 3),
                    )
                    if ni == 3:
                        mm.then_inc(psem, 1)
                if c == G - 1:
                    # tail: W_B^T est[:, last frame] -> psT[:, 1] (width 2:
                    # fp32r matmuls need an even moving width; col 0 is junk)
                    for ni in range(4):
                        mm = nc.tensor.matmul(
                            psT[:], wt_sb[:, ni, STEP:L], eb[b][:, ni, w - 1 : w + 1],
                            start=(ni == 0), stop=(ni == 3),
                        )
                        if ni == 3:
                            mm.then_inc(psem, 1)
                if g >= 2 and g % 2 == 0:
                    transposes((g - 2) // 2)

            loop_or_unroll(W, ET.PE, chunk)
            if not bench:
                transposes(NQ - 1)

        @block.scalar
        def _(scalar):
            W = _Waiter(scalar)
            scalar.dma_start(wt_sb[:], wt_r).then_inc(wsem, 16)
            scalar.dma_start(id_sb[:], ident[:]).then_inc(wsem, 16)

            def ct_copy(q):
                b = q % 2
                pw = PQW[q % NQ] // NT
                W.wait(tsem, q + 1)                # T(q) done
                if q >= NCT:
                    W.wait(osem[q % NCT], 16 * (q // NCT))  # ct slot free
                nc.scalar.copy(
                    out=ct[q % NCT][0:pw, :], in_=pst[b][0:pw, :]
                ).then_inc(ctsem, 1)

            def out_dma(q):
                qn = q % NQ
                qw = PQW[qn]
                pw = qw // NT
                f = starts[2 * qn]
                dst = out[8 * f : 8 * f + 8 * qw].rearrange(
                    "(p m j) -> p m j", p=pw, m=NT
                )
                # the DMA trigger is async wrt the ACT pipe: gate on ctsem
                W.wait(ctsem, q + 1)
                scalar.dma_start(
                    dst, ct[q % NCT][0:pw, :].rearrange("p (m j) -> p m j", m=NT)
                ).then_inc(osem[q % NCT], 16)

            def chunk(g):
                c = g % G
                w = WIDTHS[c]
                bq = (g // 2) % 2                  # res slot (per pair)
                off = 0 if g % 2 == 0 else WIDTHS[c - 1]
                W.wait(psem, psem_after_mm(g))
                if g >= 4:
                    W.wait(tsem, g // 2 - 1)       # res[bq] free (T(q-2) done)
                nc.scalar.copy(
                    out=res[bq][:, off : off + w], in_=psO[g % 2][:, 0:w]
                ).then_inc(esem, 1)
                if c == G - 1:   # tail evac (its completion wait overlaps below)
                    W.wait(psem, psem_after_tail(g))
                    W.wait(osem_t, 16 * (g // G))  # res_tail free (prev pass)
                    nc.scalar.copy(out=res_tail[:], in_=psT[:, 1:2]).then_inc(esem, 1)
                if g >= 3 and g % 2 == 1:
                    ct_copy((g - 3) // 2)
                if g >= 4 and g % 2 == 0:
                    out_dma((g - 4) // 2)
                if bench and c == G - 1:   # 32B tail output DMA (in-loop)
                    W.wait(esem, esem_after_tail(g))
                    scalar.dma_start(
                        out[STEP * KLOC : TLOC].rearrange("(p x) -> p x", x=1),
                        res_tail[:],
                    ).then_inc(osem_t, 16)

            loop_or_unroll(W, ET.Activation, chunk)
            if not bench:
                # final DMAs split across both rings: tail + pair NQ-2 issue
                # from the idle sync engine in parallel with ct/dma(NQ-1)
                ct_copy(NQ - 1)
                out_dma(NQ - 1)

    return nc


def build_nc():
    return _build(None)


def build_bench_nc(loops):
    return _build(loops)


def audit_waits(nc, max_show=12):
    """Count on_wait entries per instruction; the TPB ISA allows ONE."""
    import json

    d = json.loads(nc.to_json_bytes())
    bad = []

    def walk(blocks):
        for bb in blocks:
            for i in bb.get("instructions", []):
                si = i.get("sync_info") or {}
                w = si.get("on_wait") or []
                if len(w) > 1:
                    bad.append(
                        (
                            i["name"],
                            i.get("opcode"),
                            len(w),
                            [s_.get("ant_name") for s_ in w],
                        )
                    )
            walk(bb.get("blocks", []))

    walk(d["functions"][0]["blocks"])
    return bad[:max_show], len(bad)


_NC_CACHE = {}


def _get_nc():
    if "v3" not in _NC_CACHE:
        _NC_CACHE["v3"] = build_nc()
    return _NC_CACHE["v3"]


def _pack_x(xx):
    """xx [2, N, KLOC] -> flat [128, TOTX]: per input block, a
    per-partition-contiguous (t, ni, k) layout incl. the 1-frame halo
    (block 0's halo = zeros)."""
    flat = np.empty((128, TOTX), dtype=np.float32)
    starts_ = np.cumsum([0] + WIDTHS).tolist()
    bstart = [1024 * j for j in range(NPAIRIN)] + [starts_[c] for c in range(12, 20)]
    for i in range(NS_IN):
        f0, hw = bstart[i], IN_HW[i]
        blk = np.empty((2, N, hw), dtype=np.float32)
        if i == 0:
            blk[:, :, 0] = 0.0
            blk[:, :, 1:] = xx[:, :, 0 : hw - 1]
        else:
            blk[:] = xx[:, :, f0 - 1 : f0 + hw - 1]
        # [2, (ni p), hw] -> [p, (t ni hw)]
        flat[:, DOFF[i] : DOFF[i + 1]] = (
            blk.reshape(2, 4, 128, hw).transpose(2, 0, 1, 3).reshape(128, 8 * hw)
        )
    return flat


def make_in_maps(mixture_w, est_mask, W):
    mixture_w = np.asarray(mixture_w, dtype=np.float32)
    est_mask = np.asarray(est_mask, dtype=np.float32)
    W = np.asarray(W, dtype=np.float32)
    # wt[p, (ni l)] = W.T[ni*128+p, l]
    wtp = np.ascontiguousarray(
        W.T.reshape(4, 128, L).transpose(1, 0, 2).reshape(128, 4 * L)
    )
    ident = np.eye(8, dtype=np.float32)
    in_maps = []
    for c in range(8):
        b, h = c // 2, c % 2
        xx = np.stack(
            [
                mixture_w[b, :, h * KLOC : (h + 1) * KLOC],
                est_mask[b, :, h * KLOC : (h + 1) * KLOC],
            ]
        )
        in_maps.append({"x": _pack_x(xx), "wt": wtp, "ident": ident})
    return in_maps


def assemble(results):
    T = STEP * (K - 1) + L
    out = np.zeros((B, T), dtype=np.float32)
    for c in range(8):
        b, h = c // 2, c % 2
        out[b, h * STEP * KLOC : h * STEP * KLOC + TLOC] += results[c]["out"]
    return out


def run(mixture_w, est_mask, W, trace=False, **spmd_kwargs):
    """Shard, run on 8 cores, gather. Returns (out, BassKernelResults)."""
    in_maps = make_in_maps(mixture_w, est_mask, W)
    nc = _get_nc()
    kr = run_bass_kernel_spmd(
        nc, in_maps, core_ids=list(range(8)), trace=trace, **spmd_kwargs
    )
    return assemble(kr.results), kr


def kernel(mixture_w, est_mask, W):
    out, _ = run(mixture_w, est_mask, W)
    return out


# revision 81
# speedup vs baseline: 1.0966x; 1.0086x over previous
"""Trainium2 Bass kernel for nn_Decoder (mask-multiply + Linear(512->16) + overlap-add).

Full-input contract: kernel(mixture_w, est_mask, W) -> [4, 128008] float32.

Sharding: 8 cores = 4 batches x 2 K-halves (8000 frames each).

Folded overlap-add: out[8m+r] = sum_n est[n,m] W[r,n] + sum_n est[n,m-1] W[r+8,n],
so both overlap-add terms accumulate into ONE psum bank per chunk: the W[0:8]
("A") matmuls run against the chunk's est window and the W[8:16] ("B") matmuls
run against the same window shifted one frame left (each input block carries a
1-frame halo; block 0's halo is pre-zeroed by the host). No DVE add, no
second psum evacuation, no cross-chunk compute dependency.

The host packs each input block [mw; em] per-partition-contiguously, so every
input DMA is 128 descriptors of up to 32KB instead of 1024 small strided
ones; output pairs share one 8-transpose group so output DMAs write 256B
contiguous runs (1 descriptor per 8 frames). Total ~2.9K DMA descriptors per
pass vs ~19K for the naive layouts.

Per-chunk pipeline (all cross-engine consumer stages run >=1 chunk behind
their producer so no semaphore round-trip sits inside one chunk period):
  SP  : wt/ident DMAs, then one input DMA per block (14 blocks/pass: six
        2-chunk blocks then eight 1-chunk blocks -- big early hides the ramp,
        small late shortens the post-stream drain)
  DVE : est = x0 * x1 (f32r out) -- the only per-chunk DVE op
  PE  : 8 accumulating matmuls -> psO[8,w] (complete output block, j-major),
        then per output pair q at chunk 2q+2: 8 transposes res -> pst[qw/8,64]
  ACT : evac psO(g)->res, ct-copy pair (g-3)/2, output-DMA issue pair (g-4)/2
Tail (8 samples = W_B^T est[:,last]) rides the last chunk's matmul group.
Host adds the 8-sample seam between the two K-halves of each batch.

Semaphores incremented by MULTIPLE DMAs are rings (dsb/dss/osem): the 16
SDMA engines inc independently, so counts from back-to-back DMAs on one
semaphore interleave and a waiter could fire before the older DMA fully
landed; ring slots + a transitive gate order same-semaphore DMAs.
Every instruction carries at most one semaphore wait (ISA limit)."""

import numpy as np

import concourse.bass as bass
import concourse.mybir as mybir
from concourse.bass_utils import run_bass_kernel_spmd

F32 = mybir.dt.float32
F32R = mybir.dt.float32r

B, N, K, L = 4, 512, 16000, 16
STEP = L // 2              # 8
KLOC = K // 2              # 8000 frames per core
TLOC = STEP * (KLOC - 1) + L   # 64008 local output samples
# frames per chunk (<=512: psum bank; %16==0 pair sums for NT=16 transpose
# groups; count divisible by the ring depths so bench-loop semaphore sites
# stay stable; the shrinking tail shortens the post-stream pipeline drain)
WIDTHS = [512] * 14 + [320, 192, 128, 96, 64, 32]
assert sum(WIDTHS) == KLOC and all(w % 4 == 0 and w <= 512 for w in WIDTHS)
WMAX = max(WIDTHS)
# output pairs: chunks (2q, 2q+1) share one transpose group of NT=32 -> the
# output DMA writes 256-float (1KB, line-rate) contiguous runs, 1
# descriptor per 32 frames
NT = 32
NQ = len(WIDTHS) // 2
PQW = [WIDTHS[2 * q] + WIDTHS[2 * q + 1] for q in range(NQ)]
assert all(p % NT == 0 and p // NT <= 128 for p in PQW)
PWMAX = max(PQW)
# input DMA blocks: chunks 0-11 as six 2-chunk blocks (2-buffer ring),
# chunks 12-19 individually (4-buffer ring; the fine tail keeps the
# post-stream drain short). The host packs each block's [mw; em] data (incl.
# 1-frame halo; block 0's halo pre-zeroed) per-partition-contiguously, so
# every input DMA is 128 descriptors of up to 33KB instead of 1024 small
# strided ones.
NPAIRIN = 6                # 2-chunk input blocks (chunks 0..11)
NSING = 8                  # single-chunk input blocks (chunks 12..19)
NS_IN = NPAIRIN + NSING    # input blocks per pass
IN_HW = [2 * 512 + 1] * NPAIRIN + [WIDTHS[c] + 1 for c in range(12, 20)]
DOFF = np.cumsum([0] + [8 * h for h in IN_HW]).tolist()
TOTX = DOFF[-1]


class _Waiter:
    """Absolute-target waits (single pass) or register-tracked targets with
    constant per-site deltas (inside a bench Fori hardware loop)."""

    def __init__(self, eng):
        self.eng = eng
        self.last = {}
        self.regs = None

    def wait_monotone(self, sem, target):
        """Clamp target up to this engine's previous wait on the same sem
        (for sites whose analytic target regresses; the stronger wait is
        safe when the producer chain never runs through this engine)."""
        if sem.name in self.last:
            target = max(target, self.last[sem.name][1])
        self.wait(sem, target)

    def wait(self, sem, target):
        if self.regs is None:
            self.eng.wait_ge(sem, target)
            self.last[sem.name] = (sem, target)
        else:
            _, prev = self.last[sem.name]
            delta = target - prev
            assert delta >= 0, (sem.name, prev, target)
            self.last[sem.name] = (sem, target)
            reg = self.regs[sem.name]
            if delta:
                self.eng.reg_add(reg, reg, delta)
            self.eng.wait_ge(sem, reg)

    def enter_loop(self):
        self.regs = {}
        for name, (sem, target) in self.last.items():
            reg = self.eng.alloc_register(f"{name}_tgt")
            self.eng.reg_mov(reg, target)
            self.regs[name] = reg


def _build(loops: int | None) -> bass.Bass:
    """loops=None -> graded single-pass kernel (absolute waits only).
    loops>=3 -> bench variant with per-engine Fori steady-state loops."""
    bench = loops is not None
    G = len(WIDTHS)                    # chunks per pass
    starts = np.cumsum([0] + WIDTHS).tolist()   # frame offset per chunk

    nc = bass.Bass()
    x = nc.dram_tensor("x", [128, TOTX], F32, kind="ExternalInput")
    # f32r has f32 storage: DMA the repacked weight straight into the f32r
    # stationary tile (full-rate PE) with no cast copy.
    wt = nc.dram_tensor("wt", [128, 4 * L], F32R, kind="ExternalInput")
    ident = nc.dram_tensor("ident", [8, 8], F32, kind="ExternalInput")
    out = nc.dram_tensor("out", [TLOC], F32, kind="ExternalOutput")

    wt_r = wt.rearrange("p (ni l) -> p ni l", ni=4)

    from contextlib import ExitStack

    with ExitStack() as stk:
        e = stk.enter_context
        xp = [e(nc.sbuf_tensor(f"xp{i}", [128, 8 * IN_HW[0]], F32)) for i in range(2)]
        xss = [e(nc.sbuf_tensor(f"xss{i}", [128, 8 * (WMAX + 1)], F32)) for i in range(4)]
        eb = [e(nc.sbuf_tensor(f"eb{i}", [128, 4, WMAX + 1], F32R)) for i in range(2)]
        wt_sb = e(nc.sbuf_tensor("wt_sb", [128, 4, L], F32R))
        id_sb = e(nc.sbuf_tensor("id_sb", [8, 8], F32))
        res = [e(nc.sbuf_tensor(f"res{i}", [8, PWMAX], F32)) for i in range(2)]
        res_tail = e(nc.sbuf_tensor("res_tail", [8, 1], F32))
        # ct ring: the out-DMA completion gate (osem) binds NCT pairs back,
        # so output DMAs queued behind multi-hundred-KB input-DMA packets
        # never stall the ACT pipeline (NQ % NCT == 0: bench site stability)
        NCT = 5
        ct = [e(nc.sbuf_tensor(f"ct{i}", [PWMAX // NT, 8 * NT], F32)) for i in range(NCT)]
        psO = [e(nc.psum_tensor(f"psO{i}", [8, WMAX], F32)) for i in range(2)]
        pst = [e(nc.psum_tensor(f"pst{i}", [PWMAX // NT, 8 * NT], F32)) for i in range(2)]
        psT = e(nc.psum_tensor("psT", [8, 2], F32))
        # Semaphores incremented by MULTIPLE DMAs must be rings: the 16 SDMA
        # engines inc independently, so counts from back-to-back DMAs on one
        # sem interleave and a waiter can fire before the older DMA fully
        # landed. Ring slot g%NX / g%NCT + a transitive gate (msem / the
        # ct-slot wait) orders same-sem DMAs.
        wsem = e(nc.semaphore("wsem"))
        dsb = [e(nc.semaphore(f"dsb{i}")) for i in range(2)]
        dss = [e(nc.semaphore(f"dss{i}")) for i in range(4)]
        msem = e(nc.semaphore("msem"))
        psem = e(nc.semaphore("psem"))
        tsem = e(nc.semaphore("tsem"))
        esem = e(nc.semaphore("esem"))
        ctsem = e(nc.semaphore("ctsem"))
        osem = [e(nc.semaphore(f"osem{i}")) for i in range(NCT)]
        osem_t = e(nc.semaphore("osem_t"))
        block = e(nc.Block())

        ET = mybir.EngineType

        # Semaphore ledger (g = global chunk index, c = g % G):
        #   mult(g) done  <=> msem = g + 1
        #   MMs(g) done   <=> psem = g + 1 + (tails of completed passes)
        #   evac(g) done  <=> esem = g + 1 + (tail evacs of completed passes)
        #   T(g) done     <=> tsem = g + 1
        #   ct(g) done    <=> ctsem = g + 1
        def psem_after_mm(g):
            return g + 1 + g // G

        def psem_after_tail(g):
            return g + 2 + g // G

        def esem_after_evac(g):
            return g + 1 + g // G

        def esem_after_tail(g):
            return g + 2 + g // G

        def loop_or_unroll(W, engine_type, chunk_fn, lo=0, hi=None):
            """Emit chunk_fn(lo..hi-1) unrolled (single pass), or peel two
            passes then Fori over the rest (bench)."""
            if not bench:
                for g in range(lo, hi if hi is not None else G):
                    chunk_fn(g)
                return
            for g in range(2 * G):
                chunk_fn(g)
            W.enter_loop()
            with nc.Fori(2, loops, engines=[engine_type]):
                for cc in range(G):
                    chunk_fn(2 * G + cc)

        @block.sync
        def _(sync):
            W = _Waiter(sync)

            def dma_block(it):
                p, j = it // NS_IN, it % NS_IN
                if j < NPAIRIN:    # 2-chunk block j (chunks 2j, 2j+1)
                    bi = NPAIRIN * p + j
                    if bi >= 2:
                        # xp[bi%2] free: mult of last chunk of block bi-2 done
                        gprev = G * ((bi - 2) // NPAIRIN) + 2 * ((bi - 2) % NPAIRIN) + 1
                        W.wait_monotone(msem, gprev + 1)
                    sync.dma_start(
                        xp[bi % 2][:], x[:, DOFF[j] : DOFF[j + 1]]
                    ).then_inc(dsb[bi % 2], 16)
                else:              # single-chunk block (chunk 12 + k)
                    k = j - NPAIRIN
                    si = NSING * p + k
                    if si >= 4:
                        # xss[si%4] free: mult of the chunk of block si-4 done
                        gprev = G * ((si - 4) // NSING) + 12 + (si - 4) % NSING
                        W.wait_monotone(msem, gprev + 1)
                    sync.dma_start(
                        xss[si % 4][:, 0 : 8 * IN_HW[j]], x[:, DOFF[j] : DOFF[j + 1]]
                    ).then_inc(dss[si % 4], 16)

            if not bench:
                for it in range(NS_IN):
                    dma_block(it)
            else:
                for it in range(2 * NS_IN):
                    dma_block(it)
                W.enter_loop()
                with nc.Fori(2, loops, engines=[ET.SP]):
                    for cc in range(NS_IN):
                        dma_block(2 * NS_IN + cc)
            if not bench:
                sync.wait_ge(esem, G + 1)   # tail evac done
                sync.dma_start(
                    out[STEP * KLOC : TLOC].rearrange("(p x) -> p x", x=1),
                    res_tail[:],
                ).then_inc(osem_t, 16)
                q_ep = NQ - 2
                qw = PQW[q_ep]
                pw = qw // NT
                f = starts[2 * q_ep]
                dst = out[8 * f : 8 * f + 8 * qw].rearrange(
                    "(p m j) -> p m j", p=pw, m=NT
                )
                sync.wait_ge(ctsem, q_ep + 1)
                sync.dma_start(
                    dst,
                    ct[q_ep % NCT][0:pw, :].rearrange("p (m j) -> p m j", m=NT),
                ).then_inc(osem[q_ep % NCT], 16)

        @block.vector
        def _(vector):
            W = _Waiter(vector)

            def chunk(g):
                c = g % G
                w = WIDTHS[c]
                b = g % 2
                p = g // G
                if c < 2 * NPAIRIN:
                    j = c // 2
                    bi = NPAIRIN * p + j
                    W.wait(dsb[bi % 2], 16 * (bi // 2 + 1))
                    buf, hw = xp[bi % 2], IN_HW[j]
                    rel = 0 if c % 2 == 0 else WIDTHS[c - 1]
                else:
                    k = c - 2 * NPAIRIN
                    si = NSING * p + k
                    W.wait(dss[si % 4], 16 * (si // 4 + 1))
                    buf, hw = xss[si % 4], IN_HW[NPAIRIN + k]
                    rel = 0
                if g >= 2:
                    # eb[b] free: last read by MMs(g-2) (+ tail MMs if g-2
                    # ended a pass)
                    if (g - 2) % G == G - 1:
                        W.wait(psem, psem_after_tail(g - 2))
                    else:
                        W.wait(psem, psem_after_mm(g - 2))
                xv = buf[:, 0 : 8 * hw].rearrange(
                    "p (t ni k) -> p t ni k", t=2, ni=4
                )
                nc.vector.tensor_mul(
                    out=eb[b][:, :, 0 : w + 1],
                    in0=xv[:, 0, :, rel : rel + w + 1],
                    in1=xv[:, 1, :, rel : rel + w + 1],
                ).then_inc(msem, 1)

            loop_or_unroll(W, ET.DVE, chunk)

        @block.tensor
        def _(tensor):
            W = _Waiter(tensor)
            tensor.wait_ge(wsem, 32)   # wt_sb + id_sb loaded

            def transposes(q):
                # pair q = chunks (2q, 2q+1): 8 transposes res -> pst[PW, 64]
                b = q % 2
                qw = PQW[q % NQ]
                pw = qw // NT
                W.wait(esem, esem_after_evac(2 * q + 1))  # pair fully evac'd
                if q >= 2:
                    W.wait(ctsem, q - 1)           # pst[b] free (ct(q-2) done)
                for m in range(NT):
                    tr = nc.tensor.transpose(
                        pst[b][0:pw, 8 * m : 8 * m + 8],
                        res[b][:, m:qw:NT],
                        id_sb[:],
                    )
                    if m == NT - 1:
                        tr.then_inc(tsem, 1)

            def chunk(g):
                c = g % G
                w = WIDTHS[c]
                b = g % 2
                W.wait(msem, g + 1)                    # mult(g) done
                if g >= 2:
                    W.wait(esem, esem_after_evac(g - 2))  # psO[b] free
                for ni in range(4):
                    nc.tensor.matmul(
                        psO[b][:, 0:w], wt_sb[:, ni, 0:STEP], eb[b][:, ni, 1 : w + 1],
                        start=(ni == 0), stop=False,
                    )
                for ni in range(4):
                    mm = nc.tensor.matmul(
                        psO[b][:, 0:w], wt_sb[:, ni, STEP:L], eb[b][:, ni, 0:w],
                        start=False, stop=(ni == 3),
                    )
                    if ni == 3:
                        mm.then_inc(psem, 1)
                if c == G - 1:
                    # tail: W_B^T est[:, last frame] -> psT[:, 1] (width 2:
                    # fp32r matmuls need an even moving width; col 0 is junk)
                    for ni in range(4):
                        mm = nc.tensor.matmul(
                            psT[:], wt_sb[:, ni, STEP:L], eb[b][:, ni, w - 1 : w + 1],
                            start=(ni == 0), stop=(ni == 3),
                        )
                        if ni == 3:
                            mm.then_inc(psem, 1)
                if g >= 2 and g % 2 == 0:
                    transposes((g - 2) // 2)

            loop_or_unroll(W, ET.PE, chunk)
            if not bench:
                transposes(NQ - 1)

        @block.scalar
        def _(scalar):
            W = _Waiter(scalar)
            scalar.dma_start(wt_sb[:], wt_r).then_inc(wsem, 16)
            scalar.dma_start(id_sb[:], ident[:]).then_inc(wsem, 16)

            def ct_copy(q):
                b = q % 2
                pw = PQW[q % NQ] // NT
                W.wait(tsem, q + 1)                # T(q) done
                if q >= NCT:
                    W.wait(osem[q % NCT], 16 * (q // NCT))  # ct slot free
                nc.scalar.copy(
                    out=ct[q % NCT][0:pw, :], in_=pst[b][0:pw, :]
                ).then_inc(ctsem, 1)

            def out_dma(q):
                qn = q % NQ
                qw = PQW[qn]
                pw = qw // NT
                f = starts[2 * qn]
                dst = out[8 * f : 8 * f + 8 * qw].rearrange(
                    "(p m j) -> p m j", p=pw, m=NT
                )
                # the DMA trigger is async wrt the ACT pipe: gate on ctsem
                W.wait(ctsem, q + 1)
                scalar.dma_start(
                    dst, ct[q % NCT][0:pw, :].rearrange("p (m j) -> p m j", m=NT)
                ).then_inc(osem[q % NCT], 16)

            def chunk(g):
                c = g % G
                w = WIDTHS[c]
                bq = (g // 2) % 2                  # res slot (per pair)
                off = 0 if g % 2 == 0 else WIDTHS[c - 1]
                W.wait(psem, psem_after_mm(g))
                if g >= 4:
                    W.wait(tsem, g // 2 - 1)       # res[bq] free (T(q-2) done)
                nc.scalar.copy(
                    out=res[bq][:, off : off + w], in_=psO[g % 2][:, 0:w]
                ).then_inc(esem, 1)
                if c == G - 1:   # tail evac (its completion wait overlaps below)
                    W.wait(psem, psem_after_tail(g))
                    W.wait(osem_t, 16 * (g // G))  # res_tail free (prev pass)
                    nc.scalar.copy(out=res_tail[:], in_=psT[:, 1:2]).then_inc(esem, 1)
                if g >= 3 and g % 2 == 1:
                    ct_copy((g - 3) // 2)
                if g >= 4 and g % 2 == 0:
                    out_dma((g - 4) // 2)
                if bench and c == G - 1:   # 32B tail output DMA (in-loop)
                    W.wait(esem, esem_after_tail(g))
                    scalar.dma_start(
                        out[STEP * KLOC : TLOC].rearrange("(p x) -> p x", x=1),
                        res_tail[:],
                    ).then_inc(osem_t, 16)

            loop_or_unroll(W, ET.Activation, chunk)
            if not bench:
                # final DMAs split across both rings: tail + pair NQ-2 issue
                # from the idle sync engine in parallel with ct/dma(NQ-1)
                ct_copy(NQ - 1)
                out_dma(NQ - 1)

    return nc


def build_nc():
    return _build(None)


def build_bench_nc(loops):
    return _build(loops)


def audit_waits(nc, max_show=12):
    """Count on_wait entries per instruction; the TPB ISA allows ONE."""
    import json

    d = json.loads(nc.to_json_bytes())
    bad = []

    def walk(blocks):
        for bb in blocks:
            for i in bb.get("instructions", []):
                si = i.get("sync_info") or {}
                w = si.get("on_wait") or []
                if len(w) > 1:
                    bad.append(
                        (
                            i["name"],
                            i.get("opcode"),
                            len(w),
                            [s_.get("ant_name") for s_ in w],
                        )
                    )
            walk(bb.get("blocks", []))

    walk(d["functions"][0]["blocks"])
    return bad[:max_show], len(bad)


_NC_CACHE = {}


def _get_nc():
    if "v3" not in _NC_CACHE:
        _NC_CACHE["v3"] = build_nc()
    return _NC_CACHE["v3"]


def _pack_x(xx):
    """xx [2, N, KLOC] -> flat [128, TOTX]: per input block, a
    per-partition-contiguous (t, ni, k) layout incl. the 1-frame halo
    (block 0's halo = zeros)."""
    flat = np.empty((128, TOTX), dtype=np.float32)
    starts_ = np.cumsum([0] + WIDTHS).tolist()
    bstart = [1024 * j for j in range(NPAIRIN)] + [starts_[c] for c in range(12, 20)]
    for i in range(NS_IN):
        f0, hw = bstart[i], IN_HW[i]
        blk = np.empty((2, N, hw), dtype=np.float32)
        if i == 0:
            blk[:, :, 0] = 0.0
            blk[:, :, 1:] = xx[:, :, 0 : hw - 1]
        else:
            blk[:] = xx[:, :, f0 - 1 : f0 + hw - 1]
        # [2, (ni p), hw] -> [p, (t ni hw)]
        flat[:, DOFF[i] : DOFF[i + 1]] = (
            blk.reshape(2, 4, 128, hw).transpose(2, 0, 1, 3).reshape(128, 8 * hw)
        )
    return flat


def make_in_maps(mixture_w, est_mask, W):
    mixture_w = np.asarray(mixture_w, dtype=np.float32)
    est_mask = np.asarray(est_mask, dtype=np.float32)
    W = np.asarray(W, dtype=np.float32)
    # wt[p, (ni l)] = W.T[ni*128+p, l]
    wtp = np.ascontiguousarray(
        W.T.reshape(4, 128, L).transpose(1, 0, 2).reshape(128, 4 * L)
    )
    ident = np.eye(8, dtype=np.float32)
    in_maps = []
    for c in range(8):
        b, h = c // 2, c % 2
        xx = np.stack(
            [
                mixture_w[b, :, h * KLOC : (h + 1) * KLOC],
                est_mask[b, :, h * KLOC : (h + 1) * KLOC],
            ]
        )
        in_maps.append({"x": _pack_x(xx), "wt": wtp, "ident": ident})
    return in_maps


def assemble(results):
    T = STEP * (K - 1) + L
    out = np.zeros((B, T), dtype=np.float32)
    for c in range(8):
        b, h = c // 2, c % 2
        out[b, h * STEP * KLOC : h * STEP * KLOC + TLOC] += results[c]["out"]
    return out


def run(mixture_w, est_mask, W, trace=False, **spmd_kwargs):
    """Shard, run on 8 cores, gather. Returns (out, BassKernelResults)."""
    in_maps = make_in_maps(mixture_w, est_mask, W)
    nc = _get_nc()
    kr = run_bass_kernel_spmd(
        nc, in_maps, core_ids=list(range(8)), trace=trace, **spmd_kwargs
    )
    return assemble(kr.results), kr


def kernel(mixture_w, est_mask, W):
    out, _ = run(mixture_w, est_mask, W)
    return out
